# revision 1
# baseline (speedup 1.0000x reference)
"""MoE BERT layer (nn_MoEBertLayer) on 8 Trainium2 NeuronCores.

Sharding: pure data parallel. B=8 samples -> 1 sample per core. The MoE
routing (per-sample expert selection) is done on the host: each core's
input map carries the weights of the expert its sample routed to, packed
into matmul-friendly tile layouts. The device kernel is a dense BERT
layer for a single [512, 768] sample. No collectives.

Kernel layout strategy (per core, S=512, H=768, FF=3072, NH=12, DH=64):
  - hidden_states shipped in both [S,H] (residual/LN side) and [H,S]
    (matmul lhsT side) layouts.
  - QT/KT computed directly in [H,S] layout (out = Wq[:,m]^T @ xT).
  - V computed in [S,H] layout, with a constant ones column appended per
    head (width 65): the attention-context matmul
    ctxU_h^T = [V_h | 1]^T @ exp(scores_h^T) then yields the softmax
    denominator as its last row for free.
  - scores_h^T = K_h Q_h^T computed per head ([Sk,Sq] layout), exp via
    ScalarE with the 1/sqrt(DH) scale fused; softmax normalization is
    applied to ctxU^T (768x512 elements instead of 12x512x512).
  - Wo/FFN2 outputs come out in [S,H] layout where LayerNorm reductions
    are free-dim reductions (bn_stats/bn_aggr).
  - One on-chip transpose x1 -> x1T (24 PE transposes) feeds the FFN.
"""

import os
import sys
import numpy as np
from contextlib import ExitStack

for _p in ("/opt/trn_rl_repo", os.path.expanduser("~/.axon_site/_ro/trn_rl_repo")):
    if os.path.isdir(_p) and _p not in sys.path:
        sys.path.append(_p)

import concourse.bass as bass
import concourse.bacc as bacc
import concourse.tile as tile
from concourse import mybir
from concourse.masks import make_identity

F32 = mybir.dt.float32
F32R = mybir.dt.float32r
AF = mybir.ActivationFunctionType

P = 128
S = 512           # sequence length (per sample)
H = 768           # hidden size
FF = 3072         # FFN intermediate
NH = 12           # attention heads
DH = 64           # head dim
HK = H // P       # 6
SQ = S // P       # 4
FK = FF // P      # 24
VW = DH + 1       # 65: V head block + ones column
N_CORES = 8
EPS = 1e-12


def _emit(ctx, tc, flags):
    nc = tc.nc
    (use_bq, use_bk, use_bv, use_bo, use_bi, use_bout,
     use_mask, use_ln1, use_ln2) = flags

    xT_d = nc.dram_tensor("xT", [H, S], F32, kind="ExternalInput")
    x_d = nc.dram_tensor("x", [S, H], F32, kind="ExternalInput")
    wq_d = nc.dram_tensor("wq", [HK, P, HK, P], F32, kind="ExternalInput")
    wk_d = nc.dram_tensor("wk", [HK, P, HK, P], F32, kind="ExternalInput")
    wv_d = nc.dram_tensor("wv", [HK, P, H], F32, kind="ExternalInput")
    wo_d = nc.dram_tensor("wo", [HK, P, H], F32, kind="ExternalInput")
    wi_d = nc.dram_tensor("wi", [FK, P, HK, P], F32, kind="ExternalInput")
    wout_d = nc.dram_tensor("wout", [FK, P, H], F32, kind="ExternalInput")
    out_d = nc.dram_tensor("out", [S, H], F32, kind="ExternalOutput")

    # optional inputs (general path; absent in the fast path)
    bq_d = nc.dram_tensor("bq", [P, HK], F32, kind="ExternalInput") if use_bq else None
    bk_d = nc.dram_tensor("bk", [P, HK], F32, kind="ExternalInput") if use_bk else None
    bv_d = nc.dram_tensor("bv", [H], F32, kind="ExternalInput") if use_bv else None
    bo_d = nc.dram_tensor("bo", [H], F32, kind="ExternalInput") if use_bo else None
    bi_d = nc.dram_tensor("bi", [P, FK], F32, kind="ExternalInput") if use_bi else None
    bout_d = nc.dram_tensor("bout", [H], F32, kind="ExternalInput") if use_bout else None
    msk_d = nc.dram_tensor("msk", [P, SQ], F32, kind="ExternalInput") if use_mask else None
    ln1g_d = nc.dram_tensor("ln1g", [H], F32, kind="ExternalInput") if use_ln1 else None
    ln1b_d = nc.dram_tensor("ln1b", [H], F32, kind="ExternalInput") if use_ln1 else None
    ln2g_d = nc.dram_tensor("ln2g", [H], F32, kind="ExternalInput") if use_ln2 else None
    ln2b_d = nc.dram_tensor("ln2b", [H], F32, kind="ExternalInput") if use_ln2 else None

    def bcast_dram_row(dram_ap, parts=P):
        # DRAM [N] -> partition-broadcast [parts, N] AP for DMA
        return bass.AP(tensor=dram_ap.tensor, offset=dram_ap.offset,
                       ap=[[0, parts]] + list(dram_ap.ap))

    # ---------------- pools: whole-kernel lifetime ----------------
    const = ctx.enter_context(tc.tile_pool(name="const", bufs=1))
    wsmall = ctx.enter_context(tc.tile_pool(name="wsmall", bufs=5))
    wbig = ctx.enter_context(tc.tile_pool(name="wbig", bufs=6))
    wraw = ctx.enter_context(tc.tile_pool(name="wraw", bufs=3))
    smalls = ctx.enter_context(tc.tile_pool(name="smalls", bufs=4))

    # All matmul operands are float32r (single-pass PE mode, ~2^-12
    # rounding, 4x faster than true fp32). The BIR verifier requires a
    # rounding-capable producer, so DMA'd tensors go through a GpSimd
    # (otherwise idle) round-copy; on-chip operands are written as f32r
    # directly by their eviction op.
    def load_rounded(shape, dram_ap, tag, bufs=None, eng=None):
        raw = wraw.tile(shape, F32, tag="wraw", name="wraw")
        nc.sync.dma_start(out=raw, in_=dram_ap)
        pool = wsmall if shape[-1] == P else wbig
        t = pool.tile(shape, F32R, tag=tag, name=tag, bufs=bufs)
        if eng is nc.scalar:
            nc.scalar.copy(t, raw)
        else:
            (eng or nc.vector).tensor_copy(t, raw)
        return t

    ident = const.tile([P, P], F32)
    make_identity(nc, ident)
    eps_t = const.tile([P, 1], F32)
    nc.vector.memset(eps_t, EPS)

    bq_sb = bk_sb = bi_sb = None
    bv_bc = bo_bc = bout_bc = msk_sb = None
    ln1g_bc = ln1b_bc = ln2g_bc = ln2b_bc = None
    if use_bq:
        bq_sb = const.tile([P, HK], F32)
        nc.sync.dma_start(out=bq_sb, in_=bq_d[:])
    if use_bk:
        bk_sb = const.tile([P, HK], F32)
        nc.sync.dma_start(out=bk_sb, in_=bk_d[:])
    if use_bi:
        bi_sb = const.tile([P, FK], F32)
        nc.sync.dma_start(out=bi_sb, in_=bi_d[:])
    if use_bv:
        bv_bc = const.tile([P, H], F32)
        nc.sync.dma_start(out=bv_bc, in_=bcast_dram_row(bv_d[:]))
    if use_bo:
        bo_bc = const.tile([P, H], F32)
        nc.sync.dma_start(out=bo_bc, in_=bcast_dram_row(bo_d[:]))
    if use_bout:
        bout_bc = const.tile([P, H], F32)
        nc.sync.dma_start(out=bout_bc, in_=bcast_dram_row(bout_d[:]))
    if use_mask:
        msk_sb = const.tile([P, SQ], F32)
        nc.sync.dma_start(out=msk_sb, in_=msk_d[:])
    if use_ln1:
        ln1g_bc = const.tile([P, H], F32)
        nc.sync.dma_start(out=ln1g_bc, in_=bcast_dram_row(ln1g_d[:]))
        ln1b_bc = const.tile([P, H], F32)
        nc.sync.dma_start(out=ln1b_bc, in_=bcast_dram_row(ln1b_d[:]))
    if use_ln2:
        ln2g_bc = const.tile([P, H], F32)
        nc.sync.dma_start(out=ln2g_bc, in_=bcast_dram_row(ln2g_d[:]))
        ln2b_bc = const.tile([P, H], F32)
        nc.sync.dma_start(out=ln2b_bc, in_=bcast_dram_row(ln2b_d[:]))

    # residual + layer-norm, wave-structured: the residual add computes
    # sum(a) via its accumulator (DVE), ACT's Square computes sum(a^2);
    # var = E[a^2] - mu^2. Splits stats across DVE/ACT, no bn_stats.
    def resid_layernorm_wave(ps_l, resid_l, ebias, a_pool, dst_l,
                             g_bc, b_bc, use_gb):
        n = len(ps_l)
        a_l, as_l, sq_l, rs_l, nm_l, mu_l = [], [], [], [], [], []
        for i in range(n):
            a = a_pool.tile([P, H], F32, tag="a", bufs=4, name="a")
            asum = smalls.tile([P, 1], F32, tag="lnas", name="asum", bufs=4)
            if ebias is None:
                nc.vector.scalar_tensor_tensor(
                    a, ps_l[i], 1.0, resid_l[i], mybir.AluOpType.mult,
                    mybir.AluOpType.add, accum_out=asum)
            else:
                nc.vector.tensor_add(a, ps_l[i], resid_l[i])
                nc.vector.scalar_tensor_tensor(
                    a, a, 1.0, ebias, mybir.AluOpType.mult,
                    mybir.AluOpType.add, accum_out=asum)
            a_l.append(a); as_l.append(asum)
        for i in range(n):
            trash = smalls.tile([P, H], F32, tag="lntr", name="trash", bufs=1)
            sqsum = smalls.tile([P, 1], F32, tag="lnsq", name="sqsum", bufs=4)
            nc.scalar.activation(trash, a_l[i], AF.Square, accum_out=sqsum)
            sq_l.append(sqsum)
        for i in range(n):
            mu = smalls.tile([P, 1], F32, tag="lnmu", name="mu", bufs=4)
            nc.vector.tensor_scalar_mul(mu, as_l[i], 1.0 / H)
            var = smalls.tile([P, 1], F32, tag="lnvar", name="var", bufs=4)
            # var = sqsum/H - mu*mu
            nc.vector.tensor_mul(var, mu, mu)
            nc.vector.scalar_tensor_tensor(
                var, sq_l[i], 1.0 / H, var, mybir.AluOpType.mult,
                mybir.AluOpType.subtract)
            mu_l.append(mu)
            sd = smalls.tile([P, 1], F32, tag="lnsd", name="sd", bufs=4)
            nc.scalar.activation(sd, var, AF.Sqrt, bias=eps_t)
            rsig = smalls.tile([P, 1], F32, tag="lnrs", name="rsig", bufs=4)
            nc.vector.reciprocal(rsig, sd)
            rs_l.append(rsig)
        for i in range(n):
            nm = smalls.tile([P, 1], F32, tag="lnnm", name="nm", bufs=4)
            nc.vector.tensor_scalar(nm, mu_l[i], rs_l[i], -1.0,
                                    mybir.AluOpType.mult, mybir.AluOpType.mult)
            nm_l.append(nm)
        for i in range(n):
            nc.scalar.activation(dst_l[i], a_l[i], AF.Identity,
                                 bias=nm_l[i], scale=rs_l[i])
        if use_gb:
            for i in range(n):
                nc.vector.tensor_mul(dst_l[i], dst_l[i], g_bc)
                nc.vector.tensor_add(dst_l[i], dst_l[i], b_bc)

    # ---------------- mid-lifetime activations ----------------
    act1 = ctx.enter_context(tc.tile_pool(name="act1", bufs=1))
    x1_sb = act1.tile([P, SQ, H], F32)      # LN1 output [S,H]
    x1t_sb = act1.tile([P, HK, S], F32R)    # x1 transposed [H,S]

    with ExitStack() as phase_ab:
        actA = phase_ab.enter_context(tc.tile_pool(name="actA", bufs=1))
        x_sb = actA.tile([P, SQ, H], F32)
        xTr_sb = actA.tile([P, HK, S], F32R)
        qt_sb = actA.tile([P, HK, S], F32R)
        kt_sb = actA.tile([P, HK, S], F32R)
        vt_sb = actA.tile([P, SQ, NH * VW], F32R)
        ctxt_sb = actA.tile([P, HK, S], F32R)

        # prefetch the first QT weight column before the xT loads so the
        # first matmul's operands arrive together
        wq0_t = load_rounded([P, HK, P], wq_d[0], "wsm")
        wk0_t = load_rounded([P, HK, P], wk_d[0], "wsm")
        for m in range(HK):
            raw = wraw.tile([P, S], F32, tag="wraw", name="wraw")
            nc.sync.dma_start(out=raw, in_=xT_d[m * P:(m + 1) * P, :])
            nc.vector.tensor_copy(xTr_sb[:, m, :], raw)
        ph_att = phase_ab.enter_context(ExitStack())
        psAB = ph_att.enter_context(tc.tile_pool(name="psAB", bufs=1, space="PSUM"))
        expp = ph_att.enter_context(tc.tile_pool(name="expp", bufs=2))
        rbp = ph_att.enter_context(tc.tile_pool(name="rbp", bufs=2))

        # ---- QT / KT:  out[m] = W[:, m-block]^T @ xT  ([H,S] layout) ----
        for (w_d, dst, b_sb, useb) in ((wq_d, qt_sb, bq_sb, use_bq),
                                       (wk_d, kt_sb, bk_sb, use_bk)):
            for m in range(HK):
                if m == 0:
                    w_t = wq0_t if w_d is wq_d else wk0_t
                else:
                    w_t = load_rounded([P, HK, P], w_d[m], "wsm")
                ps = psAB.tile([P, S], F32, tag="ctx", bufs=4, name="psqk")
                for k in range(HK):
                    nc.tensor.matmul(ps, lhsT=w_t[:, k, :], rhs=xTr_sb[:, k, :],
                                     start=(k == 0), stop=(k == HK - 1))
                if useb:
                    nc.scalar.activation(dst[:, m, :], ps, AF.Identity,
                                         bias=b_sb[:, m:m + 1])
                else:
                    nc.scalar.copy(dst[:, m, :], ps)

        for m in range(SQ):
            nc.sync.dma_start(out=x_sb[:, m, :], in_=x_d[m * P:(m + 1) * P, :])

        # ---- V in [S,H] layout with ones column per head -> vt_sb ----
        ones_t = const.tile([P, NH], F32)
        nc.vector.memset(ones_t, 1.0)
        vt_v = vt_sb.rearrange("p m (h c) -> p m h c", c=VW)
        for m in range(SQ):
            nc.vector.tensor_copy(
                vt_v[:, m, :, DH:DH + 1],
                ones_t.rearrange("p (h o) -> p h o", o=1))
        wv_ts = [load_rounded([P, H], wv_d[k], "wvo", bufs=HK)
                 for k in range(HK)]
        for m in range(SQ):
            ps2 = psAB.tile([P, 2 * S], F32, tag="spair", bufs=2, name="psv")
            ps = ps2[:, 0:H]
            for k in range(HK):
                nc.tensor.matmul(ps[:, 0:512], lhsT=xTr_sb[:, k, m * P:(m + 1) * P],
                                 rhs=wv_ts[k][:, 0:512],
                                 start=(k == 0), stop=(k == HK - 1))
            for k in range(HK):
                nc.tensor.matmul(ps[:, 512:H], lhsT=xTr_sb[:, k, m * P:(m + 1) * P],
                                 rhs=wv_ts[k][:, 512:H],
                                 start=(k == 0), stop=(k == HK - 1))
            dst = vt_sb.rearrange("p m (h c) -> p m h c", c=VW)[:, m, :, 0:DH]
            src = ps.rearrange("p (h d) -> p h d", d=DH)
            if use_bv:
                nc.vector.tensor_add(
                    src, src, bv_bc.rearrange("p (h d) -> p h d", d=DH))
            nc.vector.tensor_copy(dst, src)

        # ---- attention, head pairs: the two heads of a pair live at
        # partition bases 0/64 of the same kt/qt tile, so their score
        # matmuls (K=64) row-pack onto disjoint PE row groups and run
        # concurrently; both score outputs share one 2-bank psum tile so
        # exp processes 1024 columns per ACT op. ----
        for hp in range(NH // 2):
            mt = hp
            est_l = []
            for sk in range(SQ):
                ps_s = psAB.tile([P, 2 * S], F32, tag="spair", bufs=2)
                for half in range(2):
                    pb = 64 * half
                    nc.tensor.matmul(
                        ps_s[:, half * S:(half + 1) * S],
                        lhsT=kt_sb[pb:pb + DH, mt, sk * P:(sk + 1) * P],
                        rhs=qt_sb[pb:pb + DH, mt, :],
                        start=True, stop=True)
                est = expp.tile([P, 2 * S], F32R, tag="est", bufs=5, name="est")
                if use_mask:
                    nc.scalar.activation(est, ps_s, AF.Exp,
                                         bias=msk_sb[:, sk:sk + 1], scale=0.125)
                else:
                    nc.scalar.activation(est, ps_s, AF.Exp, scale=0.125)
                est_l.append(est)
            ps_cs = []
            for half in range(2):
                h = 2 * hp + half
                ps_c = psAB.tile([P, S], F32, tag="ctx", bufs=4, name="psc")
                for sk in range(SQ):
                    nc.tensor.matmul(ps_c[0:VW, :],
                                     lhsT=vt_sb[:, sk, h * VW:(h + 1) * VW],
                                     rhs=est_l[sk][:, half * S:(half + 1) * S],
                                     start=(sk == 0), stop=(sk == SQ - 1))
                ps_cs.append(ps_c)
            # softmax normalization for the pair: gather both sums rows,
            # one GpSimd partition-broadcast, one wide fast-reciprocal
            # (a lane-starved [1,512] exact reciprocal costs ~2µs).
            srow2 = smalls.tile([1, 2 * S], F32, tag="srow")
            for half in range(2):
                nc.vector.tensor_copy(srow2[:, half * S:(half + 1) * S],
                                      ps_cs[half][DH:VW, :])
            rb = rbp.tile([P, 2 * S], F32, tag="rb")
            nc.gpsimd.partition_broadcast(rb, srow2)
            nc.vector.reciprocal_approx_fast(rb, rb)
            for half in range(2):
                pb = 64 * half
                # fused eviction + softmax normalization
                nc.vector.tensor_tensor(
                    ctxt_sb[pb:pb + DH, mt, :], ps_cs[half][0:DH, :],
                    rb[pb:pb + DH, half * S:(half + 1) * S],
                    mybir.AluOpType.mult)

        # ---- Wo + residual + LN1 ; x1 transpose ----
        # stays inside the attention psum scope (tags reused) so Wo
        # matmuls for early ctxt k-tiles interleave with late attention
        if True:
            psC = psAB
            a_pool = actA
            wo_ts = [load_rounded([P, H], wo_d[k], "wvo", bufs=HK, eng=nc.scalar)
                     for k in range(HK)]
            ps_l = []
            for m in range(SQ):
                ps2w = psC.tile([P, 2 * S], F32, tag="spair", bufs=2,
                                name="psw")
                ps = ps2w[:, 0:H]
                for k in range(HK):
                    nc.tensor.matmul(ps[:, 0:512],
                                     lhsT=ctxt_sb[:, k, m * P:(m + 1) * P],
                                     rhs=wo_ts[k][:, 0:512],
                                     start=(k == 0), stop=(k == HK - 1))
                for k in range(HK):
                    nc.tensor.matmul(ps[:, 512:H],
                                     lhsT=ctxt_sb[:, k, m * P:(m + 1) * P],
                                     rhs=wo_ts[k][:, 512:H],
                                     start=(k == 0), stop=(k == HK - 1))
                ps_l.append(ps)
            resid_layernorm_wave(ps_l, [x_sb[:, i, :] for i in range(SQ)],
                                 bo_bc if use_bo else None, actA,
                                 [x1_sb[:, i, :] for i in range(SQ)],
                                 ln1g_bc, ln1b_bc, use_ln1)
            for mm in range(SQ):
                for kb in range(HK):
                    ps_t = psC.tile([P, P], F32, tag="ctx", bufs=4, name="pst")
                    nc.tensor.transpose(
                        ps_t, x1_sb[:, mm, kb * P:(kb + 1) * P], ident)
                    nc.vector.tensor_copy(
                        x1t_sb[:, kb, mm * P:(mm + 1) * P], ps_t)

    # ---- FFN ----
    with ExitStack() as phase_ffn:
        actF = phase_ffn.enter_context(tc.tile_pool(name="actF", bufs=1))
        a_pool = phase_ffn.enter_context(tc.tile_pool(name="a_pool2", bufs=1))
        outp = phase_ffn.enter_context(tc.tile_pool(name="outp", bufs=1))
        hmidt_sb = actF.tile([P, FK, S], F32R)

        with tc.tile_pool(name="psD", bufs=4, space="PSUM") as psD:
            for m in range(FK):
                wi_t = load_rounded([P, HK, P], wi_d[m], "wsm")
                ps = psD.tile([P, S], F32, tag="f1")
                for k in range(HK):
                    nc.tensor.matmul(ps, lhsT=wi_t[:, k, :], rhs=x1t_sb[:, k, :],
                                     start=(k == 0), stop=(k == HK - 1))
                if use_bi:
                    nc.scalar.activation(hmidt_sb[:, m, :], ps, AF.Gelu,
                                         bias=bi_sb[:, m:m + 1])
                else:
                    nc.scalar.activation(hmidt_sb[:, m, :], ps, AF.Gelu)

        with tc.tile_pool(name="psE", bufs=1, space="PSUM") as psE:
            ps_m = [psE.tile([P, H], F32, tag=f"f2_{m}", bufs=1, name=f"psE{m}")
                    for m in range(SQ)]
            for k in range(FK):
                wo_t = load_rounded([P, H], wout_d[k], "wout", bufs=3)
                for m in range(SQ):
                    nc.tensor.matmul(ps_m[m][:, 0:512],
                                     lhsT=hmidt_sb[:, k, m * P:(m + 1) * P],
                                     rhs=wo_t[:, 0:512],
                                     start=(k == 0), stop=(k == FK - 1))
                    nc.tensor.matmul(ps_m[m][:, 512:H],
                                     lhsT=hmidt_sb[:, k, m * P:(m + 1) * P],
                                     rhs=wo_t[:, 512:H],
                                     start=(k == 0), stop=(k == FK - 1))
            o_l = [outp.tile([P, H], F32, tag="out", bufs=4, name="o")
                   for m in range(SQ)]
            resid_layernorm_wave(ps_m, [x1_sb[:, i, :] for i in range(SQ)],
                                 bout_bc if use_bout else None, a_pool,
                                 o_l, ln2g_bc, ln2b_bc, use_ln2)
            for m in range(SQ):
                nc.sync.dma_start(out=out_d[m * P:(m + 1) * P, :], in_=o_l[m])


_NC_CACHE = {}


def build_nc(flags):
    key = tuple(flags)
    if key not in _NC_CACHE:
        nc = bacc.Bacc("TRN2")
        with ExitStack() as ctx:
            tc = ctx.enter_context(tile.TileContext(nc))
            _emit(ctx, tc, flags)
        nc.compile()
        _NC_CACHE[key] = nc
    return _NC_CACHE[key]


def _pack_lhsT(A, mt):
    # A [in, mt*P] -> [mt, P, in//P, P] tiles: out[m, p, k, f] = A[P*k+p, P*m+f]
    kt = A.shape[0] // P
    return np.ascontiguousarray(
        A.reshape(kt, P, mt, P).transpose(2, 1, 0, 3))


def kernel(**inputs):
    hs = np.ascontiguousarray(np.asarray(inputs["hidden_states"], dtype=np.float32))
    eidx = np.asarray(inputs["expert_idx"]).astype(np.int64)
    mask = np.asarray(inputs["attention_mask"], dtype=np.float32)
    Wq = np.asarray(inputs["Wq"], dtype=np.float32)
    bq = np.asarray(inputs["bq"], dtype=np.float32)
    Wk = np.asarray(inputs["Wk"], dtype=np.float32)
    bk = np.asarray(inputs["bk"], dtype=np.float32)
    Wv = np.asarray(inputs["Wv"], dtype=np.float32)
    bv = np.asarray(inputs["bv"], dtype=np.float32)
    Wo = np.asarray(inputs["Wo"], dtype=np.float32)
    bo = np.asarray(inputs["bo"], dtype=np.float32)
    ln1_g = np.asarray(inputs["ln1_g"], dtype=np.float32)
    ln1_b = np.asarray(inputs["ln1_b"], dtype=np.float32)
    Wi = np.asarray(inputs["Wi"], dtype=np.float32)
    bi = np.asarray(inputs["bi"], dtype=np.float32)
    Wout = np.asarray(inputs["Wout"], dtype=np.float32)
    bout = np.asarray(inputs["bout"], dtype=np.float32)
    ln2_g = np.asarray(inputs["ln2_g"], dtype=np.float32)
    ln2_b = np.asarray(inputs["ln2_b"], dtype=np.float32)

    B = hs.shape[0]
    assert hs.shape == (B, S, H) and B == N_CORES

    use_bq = bool(np.any(bq))
    use_bk = bool(np.any(bk))
    use_bv = bool(np.any(bv))
    use_bo = bool(np.any(bo))
    use_bi = bool(np.any(bi))
    use_bout = bool(np.any(bout))
    use_mask = bool(np.any(mask))
    use_ln1 = bool(np.any(ln1_g != 1.0) or np.any(ln1_b))
    use_ln2 = bool(np.any(ln2_g != 1.0) or np.any(ln2_b))
    flags = (use_bq, use_bk, use_bv, use_bo, use_bi, use_bout,
             use_mask, use_ln1, use_ln2)

    nc = build_nc(flags)

    in_maps = []
    for b in range(B):
        e = int(eidx[b])
        xb = hs[b]
        im = {
            "x": xb,
            "xT": np.ascontiguousarray(xb.T),
            "wq": _pack_lhsT(Wq[e], HK),
            "wk": _pack_lhsT(Wk[e], HK),
            "wv": np.ascontiguousarray(Wv[e].reshape(HK, P, H)),
            "wo": np.ascontiguousarray(Wo[e].reshape(HK, P, H)),
            "wi": _pack_lhsT(Wi[e], FK),
            "wout": np.ascontiguousarray(Wout[e].reshape(FK, P, H)),
        }
        if use_bq:
            im["bq"] = np.ascontiguousarray(bq[e].reshape(HK, P).T)
        if use_bk:
            im["bk"] = np.ascontiguousarray(bk[e].reshape(HK, P).T)
        if use_bv:
            im["bv"] = bv[e]
        if use_bo:
            im["bo"] = bo[e]
        if use_bi:
            im["bi"] = np.ascontiguousarray(bi[e].reshape(FK, P).T)
        if use_bout:
            im["bout"] = bout[e]
        if use_mask:
            im["msk"] = np.ascontiguousarray(mask[b, 0, 0, :].reshape(SQ, P).T)
        if use_ln1:
            im["ln1g"] = ln1_g
            im["ln1b"] = ln1_b
        if use_ln2:
            im["ln2g"] = ln2_g
            im["ln2b"] = ln2_b
        in_maps.append(im)

    from concourse.bass_utils import run_bass_kernel_spmd
    res = run_bass_kernel_spmd(nc, in_maps, core_ids=list(range(N_CORES)),
                               **RUN_KWARGS)
    global LAST_RESULTS
    LAST_RESULTS = res
    out = np.stack([res.results[b]["out"] for b in range(B)], axis=0)
    return out.astype(np.float32)


RUN_KWARGS = {}
LAST_RESULTS = None


if __name__ == "__main__":
    rng = np.random.default_rng(0)
    demo = {
        "hidden_states": rng.standard_normal((8, S, H), dtype=np.float32),
        "expert_idx": rng.integers(0, 4, size=8).astype(np.int32),
        "attention_mask": np.zeros((8, 1, 1, S), np.float32),
        "Wq": 0.02 * rng.standard_normal((4, H, H), dtype=np.float32),
        "bq": np.zeros((4, H), np.float32),
        "Wk": 0.02 * rng.standard_normal((4, H, H), dtype=np.float32),
        "bk": np.zeros((4, H), np.float32),
        "Wv": 0.02 * rng.standard_normal((4, H, H), dtype=np.float32),
        "bv": np.zeros((4, H), np.float32),
        "Wo": 0.02 * rng.standard_normal((4, H, H), dtype=np.float32),
        "bo": np.zeros((4, H), np.float32),
        "ln1_g": np.ones((H,), np.float32),
        "ln1_b": np.zeros((H,), np.float32),
        "Wi": 0.02 * rng.standard_normal((4, H, FF), dtype=np.float32),
        "bi": np.zeros((4, FF), np.float32),
        "Wout": 0.02 * rng.standard_normal((4, FF, H), dtype=np.float32),
        "bout": np.zeros((4, H), np.float32),
        "ln2_g": np.ones((H,), np.float32),
        "ln2_b": np.zeros((H,), np.float32),
    }
    out = kernel(**demo)
    print("out", out.shape, out.dtype, float(np.abs(out).mean()))



# revision 2
# speedup vs baseline: 1.1821x; 1.1821x over previous
"""MoE BERT layer (nn_MoEBertLayer) on 8 Trainium2 NeuronCores.

Sharding: pure data parallel. B=8 samples -> 1 sample per core. The MoE
routing (per-sample expert selection) is done on the host: each core's
input map carries the weights of the expert its sample routed to, packed
into matmul-friendly tile layouts and pre-converted to bf16. The device
kernel is a dense BERT layer for a single [512, 768] sample.

v2 (bf16): all matmul operands are bf16 (same 1 cycle/row PE rate as
f32r, half the DMA bytes, and no f32r round-copy CASTs which burned
53us of DVE in v1). PSUM accumulation and LN statistics stay fp32.
Engine schedule is arranged to keep the PE queue dense so the tensor
engine stays at its 2.4GHz p-state:
  - QT/KT in [H,S] layout, V in [S,H]+ones-column layout (softmax
    denominator free via the attention-context matmul's last row).
  - attention head pairs: scores (K=64, PE quadrants 0/64), exp on ACT,
    ctx matmul, softmax normalization on the eviction path
    (gpsimd bcast + fast reciprocal).
  - Wo per s-tile with the k=5 (last head pair) term deferred to the
    end of each accumulation chain, hiding the last pair's
    normalization latency.
  - LN per s-tile (split stats: DVE accumulator for sum, ACT Square for
    sum of squares), then PE transposes x1 -> x1T for the FFN.
  - FFN1 streams Wi tiles; FFN2 runs s-tile-outer against a resident
    bf16 Wout so LN2 + output DMA of tile m overlap tile m+1 matmuls.
"""

import os
import sys
import numpy as np
import ml_dtypes
from contextlib import ExitStack

for _p in ("/opt/trn_rl_repo", os.path.expanduser("~/.axon_site/_ro/trn_rl_repo")):
    if os.path.isdir(_p) and _p not in sys.path:
        sys.path.append(_p)

import concourse.bass as bass
import concourse.bacc as bacc
import concourse.tile as tile
from concourse import mybir
from concourse.masks import make_identity

F32 = mybir.dt.float32
BF = mybir.dt.bfloat16
AF = mybir.ActivationFunctionType
NPBF = ml_dtypes.bfloat16

P = 128
S = 512           # sequence length (per sample)
H = 768           # hidden size
FF = 3072         # FFN intermediate
NH = 12           # attention heads
DH = 64           # head dim
HK = H // P       # 6
SQ = S // P       # 4
FK = FF // P      # 24
VW = DH + 1       # 65: V head block + ones column
N_CORES = 8
EPS = 1e-12


def _emit(ctx, tc, flags):
    nc = tc.nc
    (use_bq, use_bk, use_bv, use_bo, use_bi, use_bout,
     use_mask, use_ln1, use_ln2) = flags

    xT_d = nc.dram_tensor("xT", [H, S], BF, kind="ExternalInput")
    x_d = nc.dram_tensor("x", [S, H], BF, kind="ExternalInput")
    wq_d = nc.dram_tensor("wq", [HK, P, HK, P], BF, kind="ExternalInput")
    wk_d = nc.dram_tensor("wk", [HK, P, HK, P], BF, kind="ExternalInput")
    wv_d = nc.dram_tensor("wv", [HK, P, H], BF, kind="ExternalInput")
    wo_d = nc.dram_tensor("wo", [HK, P, H], BF, kind="ExternalInput")
    wi_d = nc.dram_tensor("wi", [FK, P, HK, P], BF, kind="ExternalInput")
    wout_d = nc.dram_tensor("wout", [FK, P, H], BF, kind="ExternalInput")
    out_d = nc.dram_tensor("out", [S, H], F32, kind="ExternalOutput")

    # optional inputs (general path; absent in the fast path)
    bq_d = nc.dram_tensor("bq", [P, HK], F32, kind="ExternalInput") if use_bq else None
    bk_d = nc.dram_tensor("bk", [P, HK], F32, kind="ExternalInput") if use_bk else None
    bv_d = nc.dram_tensor("bv", [H], F32, kind="ExternalInput") if use_bv else None
    bo_d = nc.dram_tensor("bo", [H], F32, kind="ExternalInput") if use_bo else None
    bi_d = nc.dram_tensor("bi", [P, FK], F32, kind="ExternalInput") if use_bi else None
    bout_d = nc.dram_tensor("bout", [H], F32, kind="ExternalInput") if use_bout else None
    msk_d = nc.dram_tensor("msk", [P, SQ], F32, kind="ExternalInput") if use_mask else None
    ln1g_d = nc.dram_tensor("ln1g", [H], F32, kind="ExternalInput") if use_ln1 else None
    ln1b_d = nc.dram_tensor("ln1b", [H], F32, kind="ExternalInput") if use_ln1 else None
    ln2g_d = nc.dram_tensor("ln2g", [H], F32, kind="ExternalInput") if use_ln2 else None
    ln2b_d = nc.dram_tensor("ln2b", [H], F32, kind="ExternalInput") if use_ln2 else None

    def bcast_dram_row(dram_ap, parts=P):
        # DRAM [N] -> partition-broadcast [parts, N] AP for DMA
        return bass.AP(tensor=dram_ap.tensor, offset=dram_ap.offset,
                       ap=[[0, parts]] + list(dram_ap.ap))

    # ---------------- pools: whole-kernel lifetime ----------------
    const = ctx.enter_context(tc.tile_pool(name="const", bufs=1))
    wres = ctx.enter_context(tc.tile_pool(name="wres", bufs=1))
    wstream = ctx.enter_context(tc.tile_pool(name="wstream", bufs=1))
    acts = ctx.enter_context(tc.tile_pool(name="acts", bufs=1))
    apool = ctx.enter_context(tc.tile_pool(name="apool", bufs=1))
    smalls = ctx.enter_context(tc.tile_pool(name="smalls", bufs=4))
    expp = ctx.enter_context(tc.tile_pool(name="expp", bufs=1))
    rbp = ctx.enter_context(tc.tile_pool(name="rbp", bufs=2))
    outp = ctx.enter_context(tc.tile_pool(name="outp", bufs=1))

    ident = const.tile([P, P], BF)
    make_identity(nc, ident)
    eps_t = const.tile([P, 1], F32)
    nc.vector.memset(eps_t, EPS)
    ones_t = const.tile([P, NH], BF)
    nc.vector.memset(ones_t, 1.0)

    bq_sb = bk_sb = bi_sb = None
    bv_bc = bo_bc = bout_bc = msk_sb = None
    ln1g_bc = ln1b_bc = ln2g_bc = ln2b_bc = None
    if use_bq:
        bq_sb = const.tile([P, HK], F32)
        nc.sync.dma_start(out=bq_sb, in_=bq_d[:])
    if use_bk:
        bk_sb = const.tile([P, HK], F32)
        nc.sync.dma_start(out=bk_sb, in_=bk_d[:])
    if use_bi:
        bi_sb = const.tile([P, FK], F32)
        nc.sync.dma_start(out=bi_sb, in_=bi_d[:])
    if use_bv:
        bv_bc = const.tile([P, H], F32)
        nc.sync.dma_start(out=bv_bc, in_=bcast_dram_row(bv_d[:]))
    if use_bo:
        bo_bc = const.tile([P, H], F32)
        nc.sync.dma_start(out=bo_bc, in_=bcast_dram_row(bo_d[:]))
    if use_bout:
        bout_bc = const.tile([P, H], F32)
        nc.sync.dma_start(out=bout_bc, in_=bcast_dram_row(bout_d[:]))
    if use_mask:
        msk_sb = const.tile([P, SQ], F32)
        nc.sync.dma_start(out=msk_sb, in_=msk_d[:])
    if use_ln1:
        ln1g_bc = const.tile([P, H], F32)
        nc.sync.dma_start(out=ln1g_bc, in_=bcast_dram_row(ln1g_d[:]))
        ln1b_bc = const.tile([P, H], F32)
        nc.sync.dma_start(out=ln1b_bc, in_=bcast_dram_row(ln1b_d[:]))
    if use_ln2:
        ln2g_bc = const.tile([P, H], F32)
        nc.sync.dma_start(out=ln2g_bc, in_=bcast_dram_row(ln2g_d[:]))
        ln2b_bc = const.tile([P, H], F32)
        nc.sync.dma_start(out=ln2b_bc, in_=bcast_dram_row(ln2b_d[:]))

    # ---------------- persistent activations / weights ----------------
    xTr_sb = acts.tile([P, HK, S], BF)
    x_sb = acts.tile([P, SQ, H], BF)
    qt_sb = acts.tile([P, HK, S], BF)
    kt_sb = acts.tile([P, HK, S], BF)
    vt_sb = acts.tile([P, SQ, NH * VW], BF)
    ctxt_sb = acts.tile([P, HK, S], BF)
    x1_sb = acts.tile([P, SQ, H], BF)
    x1t_sb = acts.tile([P, HK, S], BF)
    hmidt_sb = acts.tile([P, FK, S], BF)

    wv_t = wres.tile([P, HK, H], BF)
    wo_t = wres.tile([P, HK, H], BF)
    wout_t = wres.tile([P, FK, H], BF)

    # input DMA prefetch (sync queue order = arrival order)
    for m in range(HK):
        nc.sync.dma_start(out=xTr_sb[:, m, :], in_=xT_d[m * P:(m + 1) * P, :])
    # wout on the (otherwise idle at start) gpsimd queue so it streams in
    # the background without blocking the sync queue
    for k in range(FK):
        nc.gpsimd.dma_start(out=wout_t[:, k, :], in_=wout_d[k])

    # residual + layer-norm for one [P, H] tile. Split stats: the
    # residual add computes sum(a) via the DVE accumulator, ACT's Square
    # computes sum(a^2); var = E[a^2] - mu^2.
    def resid_layernorm_tile(ps, resid, ebias, dst, g_bc, b_bc, use_gb,
                             dst_bf=None):
        a = apool.tile([P, H], F32, tag="a", bufs=4, name="a")
        asum = smalls.tile([P, 1], F32, tag="lnas", name="asum", bufs=4)
        if ebias is None:
            nc.vector.scalar_tensor_tensor(
                a, ps, 1.0, resid, mybir.AluOpType.mult,
                mybir.AluOpType.add, accum_out=asum)
        else:
            nc.vector.tensor_add(a, ps, resid)
            nc.vector.scalar_tensor_tensor(
                a, a, 1.0, ebias, mybir.AluOpType.mult,
                mybir.AluOpType.add, accum_out=asum)
        trash = smalls.tile([P, H], F32, tag="lntr", name="trash", bufs=1)
        sqsum = smalls.tile([P, 1], F32, tag="lnsq", name="sqsum", bufs=4)
        nc.scalar.activation(trash, a, AF.Square, accum_out=sqsum)
        mu = smalls.tile([P, 1], F32, tag="lnmu", name="mu", bufs=4)
        nc.vector.tensor_scalar_mul(mu, asum, 1.0 / H)
        var = smalls.tile([P, 1], F32, tag="lnvar", name="var", bufs=4)
        nc.vector.tensor_mul(var, mu, mu)
        nc.vector.scalar_tensor_tensor(
            var, sqsum, 1.0 / H, var, mybir.AluOpType.mult,
            mybir.AluOpType.subtract)
        sd = smalls.tile([P, 1], F32, tag="lnsd", name="sd", bufs=4)
        nc.scalar.activation(sd, var, AF.Sqrt, bias=eps_t)
        rsig = smalls.tile([P, 1], F32, tag="lnrs", name="rsig", bufs=4)
        nc.vector.reciprocal(rsig, sd)
        nm = smalls.tile([P, 1], F32, tag="lnnm", name="nm", bufs=4)
        nc.vector.tensor_scalar(nm, mu, rsig, -1.0,
                                mybir.AluOpType.mult, mybir.AluOpType.mult)
        if use_gb:
            # general path: apply in f32, then fold gamma/beta, then copy
            xf = apool.tile([P, H], F32, tag="xf", bufs=2, name="xf")
            nc.scalar.activation(xf, a, AF.Identity, bias=nm, scale=rsig)
            nc.vector.tensor_mul(xf, xf, g_bc)
            nc.vector.tensor_add(xf, xf, b_bc)
            nc.vector.tensor_copy(dst, xf)
        else:
            nc.scalar.activation(dst, a, AF.Identity, bias=nm, scale=rsig)

    # ================ phase A: QT/KT/V ================
    with ExitStack() as phase_a:
        psA = phase_a.enter_context(tc.tile_pool(name="psA", bufs=1, space="PSUM"))

        # ---- QT / KT:  out[m] = W[:, m-block]^T @ xT  ([H,S] layout) ----
        for (w_d, dst, b_sb, useb) in ((wq_d, qt_sb, bq_sb, use_bq),
                                       (wk_d, kt_sb, bk_sb, use_bk)):
            for m in range(HK):
                w_tile = wstream.tile([P, HK, P], BF, tag="wsm", bufs=4,
                                      name="wsm")
                nc.sync.dma_start(out=w_tile, in_=w_d[m])
                ps = psA.tile([P, S], F32, tag="qk", bufs=3, name="psqk")
                for k in range(HK):
                    nc.tensor.matmul(ps, lhsT=w_tile[:, k, :],
                                     rhs=xTr_sb[:, k, :],
                                     start=(k == 0), stop=(k == HK - 1))
                if useb:
                    nc.scalar.activation(dst[:, m, :], ps, AF.Identity,
                                         bias=b_sb[:, m:m + 1])
                else:
                    nc.scalar.copy(dst[:, m, :], ps)

        # x residual + wv/wo weights arrive behind the QK weights
        for m in range(SQ):
            nc.sync.dma_start(out=x_sb[:, m, :], in_=x_d[m * P:(m + 1) * P, :])
        for k in range(HK):
            nc.sync.dma_start(out=wv_t[:, k, :], in_=wv_d[k])
        for k in range(HK):
            nc.sync.dma_start(out=wo_t[:, k, :], in_=wo_d[k])

        # ---- V in [S,H] layout with ones column per head -> vt_sb ----
        vt_v = vt_sb.rearrange("p m (h c) -> p m h c", c=VW)
        for m in range(SQ):
            nc.vector.tensor_copy(
                vt_v[:, m, :, DH:DH + 1],
                ones_t.rearrange("p (h o) -> p h o", o=1))
        for m in range(SQ):
            ps = psA.tile([P, H], F32, tag="v", bufs=2, name="psv")
            for k in range(HK):
                nc.tensor.matmul(ps[:, 0:512],
                                 lhsT=xTr_sb[:, k, m * P:(m + 1) * P],
                                 rhs=wv_t[:, k, 0:512],
                                 start=(k == 0), stop=(k == HK - 1))
            for k in range(HK):
                nc.tensor.matmul(ps[:, 512:H],
                                 lhsT=xTr_sb[:, k, m * P:(m + 1) * P],
                                 rhs=wv_t[:, k, 512:H],
                                 start=(k == 0), stop=(k == HK - 1))
            src = ps.rearrange("p (h d) -> p h d", d=DH)
            if use_bv:
                nc.vector.tensor_add(
                    src, src, bv_bc.rearrange("p (h d) -> p h d", d=DH))
            nc.vector.tensor_copy(vt_v[:, m, :, 0:DH], src)

    # ================ phase B: attention + Wo + LN1 + transpose ========
    with ExitStack() as phase_b:
        psB = phase_b.enter_context(tc.tile_pool(name="psB", bufs=1, space="PSUM"))

        # ---- attention, head pairs: heads 2hp/2hp+1 at partition bases
        # 0/64 of the kt/qt tiles -> score matmuls on PE quadrants; both
        # score outputs share one 2-bank psum tile so exp processes 1024
        # columns per ACT op. ----
        for hp in range(NH // 2):
            est_l = []
            for sk in range(SQ):
                ps_s = psB.tile([P, 2 * S], F32, tag="s", bufs=2, name="pss")
                for half in range(2):
                    pb = 64 * half
                    nc.tensor.matmul(
                        ps_s[:, half * S:(half + 1) * S],
                        lhsT=kt_sb[pb:pb + DH, hp, sk * P:(sk + 1) * P],
                        rhs=qt_sb[pb:pb + DH, hp, :],
                        start=True, stop=True)
                est = expp.tile([P, 2 * S], BF, tag="est", bufs=5, name="est")
                if use_mask:
                    nc.scalar.activation(est, ps_s, AF.Exp,
                                         bias=msk_sb[:, sk:sk + 1], scale=0.125)
                else:
                    nc.scalar.activation(est, ps_s, AF.Exp, scale=0.125)
                est_l.append(est)
            ps_cs = []
            for half in range(2):
                h = 2 * hp + half
                ps_c = psB.tile([P, S], F32, tag="c", bufs=4, name="psc")
                for sk in range(SQ):
                    nc.tensor.matmul(ps_c[0:VW, :],
                                     lhsT=vt_sb[:, sk, h * VW:(h + 1) * VW],
                                     rhs=est_l[sk][:, half * S:(half + 1) * S],
                                     start=(sk == 0), stop=(sk == SQ - 1))
                ps_cs.append(ps_c)
            # softmax normalization: gather both sums rows, one GpSimd
            # partition-broadcast, one wide fast-reciprocal; fused
            # eviction applies the normalization.
            srow2 = smalls.tile([1, 2 * S], F32, tag="srow", bufs=2)
            for half in range(2):
                nc.vector.tensor_copy(srow2[:, half * S:(half + 1) * S],
                                      ps_cs[half][DH:VW, :])
            rb = rbp.tile([P, 2 * S], F32, tag="rb")
            nc.gpsimd.partition_broadcast(rb, srow2)
            nc.vector.reciprocal_approx_fast(rb, rb)
            for half in range(2):
                pb = 64 * half
                nc.vector.tensor_tensor(
                    ctxt_sb[pb:pb + DH, hp, :], ps_cs[half][0:DH, :],
                    rb[pb:pb + DH, half * S:(half + 1) * S],
                    mybir.AluOpType.mult)

        # ---- Wo + residual + LN1, s-tile pipelined; the k=5 (last head
        # pair) matmul term runs at the end of each chain so the last
        # pair's normalization latency hides behind k=0..4 work. ----
        KORD = [0, 1, 2, 3, 4, 5]

        def wo_chain(m, korder):
            ps = psB.tile([P, 2 * S], F32, tag="s", bufs=2, name="psw")
            for i, k in enumerate(korder):
                nc.tensor.matmul(ps[:, 0:512],
                                 lhsT=ctxt_sb[:, k, m * P:(m + 1) * P],
                                 rhs=wo_t[:, k, 0:512],
                                 start=(i == 0), stop=(i == HK - 1))
            for i, k in enumerate(korder):
                nc.tensor.matmul(ps[:, 512:H],
                                 lhsT=ctxt_sb[:, k, m * P:(m + 1) * P],
                                 rhs=wo_t[:, k, 512:H],
                                 start=(i == 0), stop=(i == HK - 1))
            return ps[:, 0:H]

        ps_w = {}
        # m0/m1 k0..4 first (hiding pair-5 latency), then their k5 terms
        for m in (0, 1):
            ps = psB.tile([P, 2 * S], F32, tag="s", bufs=2, name="psw")
            ps_w[m] = ps
            for half, lo, hi in ((0, 0, 512), (1, 512, H)):
                for i, k in enumerate(KORD[:5]):
                    nc.tensor.matmul(ps[:, lo:hi],
                                     lhsT=ctxt_sb[:, k, m * P:(m + 1) * P],
                                     rhs=wo_t[:, k, lo:hi],
                                     start=(i == 0), stop=False)
        for m in (0, 1):
            for half, lo, hi in ((0, 0, 512), (1, 512, H)):
                nc.tensor.matmul(ps_w[m][:, lo:hi],
                                 lhsT=ctxt_sb[:, 5, m * P:(m + 1) * P],
                                 rhs=wo_t[:, 5, lo:hi],
                                 start=False, stop=True)
        for m in (0, 1):
            resid_layernorm_tile(ps_w[m][:, 0:H], x_sb[:, m, :],
                                 bo_bc if use_bo else None,
                                 x1_sb[:, m, :], ln1g_bc, ln1b_bc, use_ln1)
        for m in (2, 3):
            ps = wo_chain(m, KORD)
            resid_layernorm_tile(ps, x_sb[:, m, :],
                                 bo_bc if use_bo else None,
                                 x1_sb[:, m, :], ln1g_bc, ln1b_bc, use_ln1)

        # ---- x1 -> x1T via PE transposes (bf16, 1 cycle/row) ----
        for m in range(SQ):
            for kb in range(HK):
                ps_t = psB.tile([P, P], BF, tag="c", bufs=4, name="pst")
                nc.tensor.transpose(
                    ps_t, x1_sb[:, m, kb * P:(kb + 1) * P], ident)
                nc.vector.tensor_copy(
                    x1t_sb[:, kb, m * P:(m + 1) * P], ps_t)

    # ================ phase C: FFN ================
    with ExitStack() as phase_c:
        psD = phase_c.enter_context(tc.tile_pool(name="psD", bufs=1, space="PSUM"))

        # ---- FFN1: hmidT[f,:] = Wi[:,f]^T @ x1T, GELU on eviction ----
        for mf in range(FK):
            wi_t = wstream.tile([P, HK, P], BF, tag="wi", bufs=6, name="wi")
            nc.sync.dma_start(out=wi_t, in_=wi_d[mf])
            ps = psD.tile([P, S], F32, tag="f1", bufs=4, name="psf1")
            for k in range(HK):
                nc.tensor.matmul(ps, lhsT=wi_t[:, k, :], rhs=x1t_sb[:, k, :],
                                 start=(k == 0), stop=(k == HK - 1))
            if use_bi:
                nc.scalar.activation(hmidt_sb[:, mf, :], ps, AF.Gelu,
                                     bias=bi_sb[:, mf:mf + 1])
            else:
                nc.scalar.activation(hmidt_sb[:, mf, :], ps, AF.Gelu)

        # ---- FFN2 s-tile-outer against resident Wout; LN2 + out DMA of
        # tile m overlap tile m+1 matmuls ----
        for m in range(SQ):
            ps = psD.tile([P, H], F32, tag="f2", bufs=2, name="psf2")
            for lo, hi in ((0, 512), (512, H)):
                for k in range(FK):
                    nc.tensor.matmul(ps[:, lo:hi],
                                     lhsT=hmidt_sb[:, k, m * P:(m + 1) * P],
                                     rhs=wout_t[:, k, lo:hi],
                                     start=(k == 0), stop=(k == FK - 1))
            o = outp.tile([P, H], F32, tag="out", bufs=2, name="o")
            resid_layernorm_tile(ps, x1_sb[:, m, :],
                                 bout_bc if use_bout else None,
                                 o, ln2g_bc, ln2b_bc, use_ln2)
            nc.sync.dma_start(out=out_d[m * P:(m + 1) * P, :], in_=o)


_NC_CACHE = {}


def build_nc(flags):
    key = tuple(flags)
    if key not in _NC_CACHE:
        nc = bacc.Bacc("TRN2")
        with ExitStack() as ctx:
            tc = ctx.enter_context(tile.TileContext(nc))
            _emit(ctx, tc, flags)
        nc.compile()
        _NC_CACHE[key] = nc
    return _NC_CACHE[key]


def _pack_lhsT(A, mt):
    # A [in, mt*P] -> [mt, P, in//P, P] tiles: out[m, p, k, f] = A[P*k+p, P*m+f]
    kt = A.shape[0] // P
    return np.ascontiguousarray(
        A.reshape(kt, P, mt, P).transpose(2, 1, 0, 3))


def _bf(a):
    return np.ascontiguousarray(np.asarray(a).astype(NPBF))


def kernel(**inputs):
    hs = np.ascontiguousarray(np.asarray(inputs["hidden_states"], dtype=np.float32))
    eidx = np.asarray(inputs["expert_idx"]).astype(np.int64)
    mask = np.asarray(inputs["attention_mask"], dtype=np.float32)
    Wq = np.asarray(inputs["Wq"], dtype=np.float32)
    bq = np.asarray(inputs["bq"], dtype=np.float32)
    Wk = np.asarray(inputs["Wk"], dtype=np.float32)
    bk = np.asarray(inputs["bk"], dtype=np.float32)
    Wv = np.asarray(inputs["Wv"], dtype=np.float32)
    bv = np.asarray(inputs["bv"], dtype=np.float32)
    Wo = np.asarray(inputs["Wo"], dtype=np.float32)
    bo = np.asarray(inputs["bo"], dtype=np.float32)
    ln1_g = np.asarray(inputs["ln1_g"], dtype=np.float32)
    ln1_b = np.asarray(inputs["ln1_b"], dtype=np.float32)
    Wi = np.asarray(inputs["Wi"], dtype=np.float32)
    bi = np.asarray(inputs["bi"], dtype=np.float32)
    Wout = np.asarray(inputs["Wout"], dtype=np.float32)
    bout = np.asarray(inputs["bout"], dtype=np.float32)
    ln2_g = np.asarray(inputs["ln2_g"], dtype=np.float32)
    ln2_b = np.asarray(inputs["ln2_b"], dtype=np.float32)

    B = hs.shape[0]
    assert hs.shape == (B, S, H) and B == N_CORES

    use_bq = bool(np.any(bq))
    use_bk = bool(np.any(bk))
    use_bv = bool(np.any(bv))
    use_bo = bool(np.any(bo))
    use_bi = bool(np.any(bi))
    use_bout = bool(np.any(bout))
    use_mask = bool(np.any(mask))
    use_ln1 = bool(np.any(ln1_g != 1.0) or np.any(ln1_b))
    use_ln2 = bool(np.any(ln2_g != 1.0) or np.any(ln2_b))
    flags = (use_bq, use_bk, use_bv, use_bo, use_bi, use_bout,
             use_mask, use_ln1, use_ln2)

    nc = build_nc(flags)

    # per-expert packed weights, converted once and reused across cores
    packed = {}
    for e in set(int(v) for v in eidx):
        packed[e] = {
            "wq": _bf(_pack_lhsT(Wq[e], HK)),
            "wk": _bf(_pack_lhsT(Wk[e], HK)),
            "wv": _bf(Wv[e].reshape(HK, P, H)),
            "wo": _bf(Wo[e].reshape(HK, P, H)),
            "wi": _bf(_pack_lhsT(Wi[e], FK)),
            "wout": _bf(Wout[e].reshape(FK, P, H)),
        }

    in_maps = []
    for b in range(B):
        e = int(eidx[b])
        xb = hs[b]
        im = {
            "x": _bf(xb),
            "xT": _bf(xb.T),
        }
        im.update(packed[e])
        if use_bq:
            im["bq"] = np.ascontiguousarray(bq[e].reshape(HK, P).T)
        if use_bk:
            im["bk"] = np.ascontiguousarray(bk[e].reshape(HK, P).T)
        if use_bv:
            im["bv"] = bv[e]
        if use_bo:
            im["bo"] = bo[e]
        if use_bi:
            im["bi"] = np.ascontiguousarray(bi[e].reshape(FK, P).T)
        if use_bout:
            im["bout"] = bout[e]
        if use_mask:
            im["msk"] = np.ascontiguousarray(mask[b, 0, 0, :].reshape(SQ, P).T)
        if use_ln1:
            im["ln1g"] = ln1_g
            im["ln1b"] = ln1_b
        if use_ln2:
            im["ln2g"] = ln2_g
            im["ln2b"] = ln2_b
        in_maps.append(im)

    from concourse.bass_utils import run_bass_kernel_spmd
    res = run_bass_kernel_spmd(nc, in_maps, core_ids=list(range(N_CORES)),
                               **RUN_KWARGS)
    global LAST_RESULTS
    LAST_RESULTS = res
    out = np.stack([res.results[b]["out"] for b in range(B)], axis=0)
    return out.astype(np.float32)


RUN_KWARGS = {}
LAST_RESULTS = None


if __name__ == "__main__":
    rng = np.random.default_rng(0)
    demo = {
        "hidden_states": rng.standard_normal((8, S, H), dtype=np.float32),
        "expert_idx": rng.integers(0, 4, size=8).astype(np.int32),
        "attention_mask": np.zeros((8, 1, 1, S), np.float32),
        "Wq": 0.02 * rng.standard_normal((4, H, H), dtype=np.float32),
        "bq": np.zeros((4, H), np.float32),
        "Wk": 0.02 * rng.standard_normal((4, H, H), dtype=np.float32),
        "bk": np.zeros((4, H), np.float32),
        "Wv": 0.02 * rng.standard_normal((4, H, H), dtype=np.float32),
        "bv": np.zeros((4, H), np.float32),
        "Wo": 0.02 * rng.standard_normal((4, H, H), dtype=np.float32),
        "bo": np.zeros((4, H), np.float32),
        "ln1_g": np.ones((H,), np.float32),
        "ln1_b": np.zeros((H,), np.float32),
        "Wi": 0.02 * rng.standard_normal((4, H, FF), dtype=np.float32),
        "bi": np.zeros((4, FF), np.float32),
        "Wout": 0.02 * rng.standard_normal((4, FF, H), dtype=np.float32),
        "bout": np.zeros((4, H), np.float32),
        "ln2_g": np.ones((H,), np.float32),
        "ln2_b": np.zeros((H,), np.float32),
    }
    out = kernel(**demo)
    print("out", out.shape, out.dtype, float(np.abs(out).mean()))


# revision 8
# speedup vs baseline: 1.2442x; 1.0526x over previous
"""MoE BERT layer (nn_MoEBertLayer) on 8 Trainium2 NeuronCores.

Sharding: pure data parallel. B=8 samples -> 1 sample per core. The MoE
routing (per-sample expert selection) is done on the host: each core's
input map carries the weights of the expert its sample routed to, packed
into matmul-friendly tile layouts and pre-converted to bf16. The device
kernel is a dense BERT layer for a single [512, 768] sample.

v2 (bf16): all matmul operands are bf16 (same 1 cycle/row PE rate as
f32r, half the DMA bytes, and no f32r round-copy CASTs which burned
53us of DVE in v1). PSUM accumulation and LN statistics stay fp32.
Engine schedule is arranged to keep the PE queue dense so the tensor
engine stays at its 2.4GHz p-state:
  - QT/KT in [H,S] layout, V in [S,H]+ones-column layout (softmax
    denominator free via the attention-context matmul's last row).
  - attention head pairs: scores (K=64, PE quadrants 0/64), exp on ACT,
    ctx matmul, softmax normalization on the eviction path
    (gpsimd bcast + fast reciprocal).
  - Wo per s-tile with the k=5 (last head pair) term deferred to the
    end of each accumulation chain, hiding the last pair's
    normalization latency.
  - LN per s-tile (split stats: DVE accumulator for sum, ACT Square for
    sum of squares), then PE transposes x1 -> x1T for the FFN.
  - FFN1 streams Wi tiles; FFN2 runs s-tile-outer against a resident
    bf16 Wout so LN2 + output DMA of tile m overlap tile m+1 matmuls.
"""

import os
import sys
import numpy as np
import ml_dtypes
from contextlib import ExitStack

for _p in ("/opt/trn_rl_repo", os.path.expanduser("~/.axon_site/_ro/trn_rl_repo")):
    if os.path.isdir(_p) and _p not in sys.path:
        sys.path.append(_p)

import concourse.bass as bass
import concourse.bacc as bacc
import concourse.tile as tile
from concourse import mybir
from concourse.masks import make_identity

F32 = mybir.dt.float32
BF = mybir.dt.bfloat16
AF = mybir.ActivationFunctionType
NPBF = ml_dtypes.bfloat16

P = 128
S = 512           # sequence length (per sample)
H = 768           # hidden size
FF = 3072         # FFN intermediate
NH = 12           # attention heads
DH = 64           # head dim
HK = H // P       # 6
SQ = S // P       # 4
FK = FF // P      # 24
VW = DH + 1       # 65: V head block + ones column
N_CORES = 8
EPS = 1e-12


def _emit(ctx, tc, flags):
    nc = tc.nc
    (use_bq, use_bk, use_bv, use_bo, use_bi, use_bout,
     use_mask, use_ln1, use_ln2) = flags

    xT_d = nc.dram_tensor("xT", [H, S], BF, kind="ExternalInput")
    x_d = nc.dram_tensor("x", [S, H], BF, kind="ExternalInput")
    wq_d = nc.dram_tensor("wq", [HK, P, HK, P], BF, kind="ExternalInput")
    wk_d = nc.dram_tensor("wk", [HK, P, HK, P], BF, kind="ExternalInput")
    wv_d = nc.dram_tensor("wv", [HK, P, H], BF, kind="ExternalInput")
    wo_d = nc.dram_tensor("wo", [HK, P, H], BF, kind="ExternalInput")
    wi_d = nc.dram_tensor("wi", [FK, P, HK, P], BF, kind="ExternalInput")
    wout_d = nc.dram_tensor("wout", [FK, P, H], BF, kind="ExternalInput")
    out_d = nc.dram_tensor("out", [S, H], F32, kind="ExternalOutput")

    # optional inputs (general path; absent in the fast path)
    bq_d = nc.dram_tensor("bq", [P, HK], F32, kind="ExternalInput") if use_bq else None
    bk_d = nc.dram_tensor("bk", [P, HK], F32, kind="ExternalInput") if use_bk else None
    bv_d = nc.dram_tensor("bv", [H], F32, kind="ExternalInput") if use_bv else None
    bo_d = nc.dram_tensor("bo", [H], F32, kind="ExternalInput") if use_bo else None
    bi_d = nc.dram_tensor("bi", [P, FK], F32, kind="ExternalInput") if use_bi else None
    bout_d = nc.dram_tensor("bout", [H], F32, kind="ExternalInput") if use_bout else None
    msk_d = nc.dram_tensor("msk", [P, SQ], F32, kind="ExternalInput") if use_mask else None
    ln1g_d = nc.dram_tensor("ln1g", [H], F32, kind="ExternalInput") if use_ln1 else None
    ln1b_d = nc.dram_tensor("ln1b", [H], F32, kind="ExternalInput") if use_ln1 else None
    ln2g_d = nc.dram_tensor("ln2g", [H], F32, kind="ExternalInput") if use_ln2 else None
    ln2b_d = nc.dram_tensor("ln2b", [H], F32, kind="ExternalInput") if use_ln2 else None

    def bcast_dram_row(dram_ap, parts=P):
        # DRAM [N] -> partition-broadcast [parts, N] AP for DMA
        return bass.AP(tensor=dram_ap.tensor, offset=dram_ap.offset,
                       ap=[[0, parts]] + list(dram_ap.ap))

    # ---------------- pools: whole-kernel lifetime ----------------
    const = ctx.enter_context(tc.tile_pool(name="const", bufs=1))
    wres = ctx.enter_context(tc.tile_pool(name="wres", bufs=1))
    wstream = ctx.enter_context(tc.tile_pool(name="wstream", bufs=1))
    acts = ctx.enter_context(tc.tile_pool(name="acts", bufs=1))
    apool = ctx.enter_context(tc.tile_pool(name="apool", bufs=1))
    smalls = ctx.enter_context(tc.tile_pool(name="smalls", bufs=4))
    expp = ctx.enter_context(tc.tile_pool(name="expp", bufs=1))
    rbp = ctx.enter_context(tc.tile_pool(name="rbp", bufs=2))
    outp = ctx.enter_context(tc.tile_pool(name="outp", bufs=1))

    ident = const.tile([P, P], BF)
    make_identity(nc, ident)
    eps_t = const.tile([P, 1], F32)
    nc.vector.memset(eps_t, EPS)
    ones_t = const.tile([P, NH], BF)
    nc.vector.memset(ones_t, 1.0)

    bq_sb = bk_sb = bi_sb = None
    bv_bc = bo_bc = bout_bc = msk_sb = None
    ln1g_bc = ln1b_bc = ln2g_bc = ln2b_bc = None
    if use_bq:
        bq_sb = const.tile([P, HK], F32)
        nc.sync.dma_start(out=bq_sb, in_=bq_d[:])
    if use_bk:
        bk_sb = const.tile([P, HK], F32)
        nc.sync.dma_start(out=bk_sb, in_=bk_d[:])
    if use_bi:
        bi_sb = const.tile([P, FK], F32)
        nc.sync.dma_start(out=bi_sb, in_=bi_d[:])
    if use_bv:
        bv_bc = const.tile([P, H], F32)
        nc.sync.dma_start(out=bv_bc, in_=bcast_dram_row(bv_d[:]))
    if use_bo:
        bo_bc = const.tile([P, H], F32)
        nc.sync.dma_start(out=bo_bc, in_=bcast_dram_row(bo_d[:]))
    if use_bout:
        bout_bc = const.tile([P, H], F32)
        nc.sync.dma_start(out=bout_bc, in_=bcast_dram_row(bout_d[:]))
    if use_mask:
        msk_sb = const.tile([P, SQ], F32)
        nc.sync.dma_start(out=msk_sb, in_=msk_d[:])
    if use_ln1:
        ln1g_bc = const.tile([P, H], F32)
        nc.sync.dma_start(out=ln1g_bc, in_=bcast_dram_row(ln1g_d[:]))
        ln1b_bc = const.tile([P, H], F32)
        nc.sync.dma_start(out=ln1b_bc, in_=bcast_dram_row(ln1b_d[:]))
    if use_ln2:
        ln2g_bc = const.tile([P, H], F32)
        nc.sync.dma_start(out=ln2g_bc, in_=bcast_dram_row(ln2g_d[:]))
        ln2b_bc = const.tile([P, H], F32)
        nc.sync.dma_start(out=ln2b_bc, in_=bcast_dram_row(ln2b_d[:]))

    # ---------------- persistent activations / weights ----------------
    xTr_sb = acts.tile([P, HK, S], BF)
    x_sb = acts.tile([P, SQ, H], BF)
    qt_sb = acts.tile([P, HK, S], BF)
    kt_sb = acts.tile([P, HK, S], BF)
    vt_sb = acts.tile([P, SQ, NH * VW], BF)
    ctxt_sb = acts.tile([P, HK, S], BF)
    x1_sb = acts.tile([P, SQ, H], BF)
    x1t_sb = acts.tile([P, HK, S], BF)
    hmidt_sb = acts.tile([P, FK, S], BF)

    wq_t = wres.tile([P, HK, HK, P], BF)
    wk_t = wres.tile([P, HK, HK, P], BF)
    wv_t = wres.tile([P, HK, H], BF)
    wo_t = wres.tile([P, HK, H], BF)
    wout_t = wres.tile([P, FK, H], BF)

    # Input DMA prefetch, strictly in consumption order on one queue so
    # early phases never contend for HBM bandwidth with late-phase
    # weights. Multi-tile tensors move as a single rearranged DMA.
    nc.sync.dma_start(out=xTr_sb, in_=xT_d[:].rearrange("(k p) s -> p k s", p=P))
    for m in range(HK):
        nc.sync.dma_start(out=wq_t[:, m, :, :], in_=wq_d[m])
    for m in range(HK):
        nc.sync.dma_start(out=wk_t[:, m, :, :], in_=wk_d[m])
    nc.sync.dma_start(out=wv_t, in_=wv_d[:].rearrange("k p h -> p k h"))
    nc.sync.dma_start(out=wo_t, in_=wo_d[:].rearrange("k p h -> p k h"))
    nc.sync.dma_start(out=x_sb, in_=x_d[:].rearrange("(m p) h -> p m h", p=P))
    nc.sync.dma_start(out=wout_t, in_=wout_d[:].rearrange("k p h -> p k h"))

    # residual + layer-norm for one [P, H] tile. Split stats: the
    # residual add computes sum(a) via the DVE accumulator, ACT's Square
    # computes sum(a^2); var = E[a^2] - mu^2.
    def resid_layernorm_tile(ps, resid, ebias, dst, g_bc, b_bc, use_gb,
                             dst_bf=None):
        a = apool.tile([P, H], F32, tag="a", bufs=4, name="a")
        asum = smalls.tile([P, 1], F32, tag="lnas", name="asum", bufs=4)
        if ebias is None:
            nc.vector.scalar_tensor_tensor(
                a, ps, 1.0, resid, mybir.AluOpType.mult,
                mybir.AluOpType.add, accum_out=asum)
        else:
            nc.vector.tensor_add(a, ps, resid)
            nc.vector.scalar_tensor_tensor(
                a, a, 1.0, ebias, mybir.AluOpType.mult,
                mybir.AluOpType.add, accum_out=asum)
        trash = smalls.tile([P, H], F32, tag="lntr", name="trash", bufs=1)
        sqsum = smalls.tile([P, 1], F32, tag="lnsq", name="sqsum", bufs=4)
        nc.scalar.activation(trash, a, AF.Square, accum_out=sqsum)
        mu = smalls.tile([P, 1], F32, tag="lnmu", name="mu", bufs=4)
        nc.vector.tensor_scalar_mul(mu, asum, 1.0 / H)
        var = smalls.tile([P, 1], F32, tag="lnvar", name="var", bufs=4)
        nc.vector.tensor_mul(var, mu, mu)
        nc.vector.scalar_tensor_tensor(
            var, sqsum, 1.0 / H, var, mybir.AluOpType.mult,
            mybir.AluOpType.subtract)
        sd = smalls.tile([P, 1], F32, tag="lnsd", name="sd", bufs=4)
        nc.scalar.activation(sd, var, AF.Sqrt, bias=eps_t)
        rsig = smalls.tile([P, 1], F32, tag="lnrs", name="rsig", bufs=4)
        nc.vector.reciprocal(rsig, sd)
        nm = smalls.tile([P, 1], F32, tag="lnnm", name="nm", bufs=4)
        nc.vector.tensor_scalar(nm, mu, rsig, -1.0,
                                mybir.AluOpType.mult, mybir.AluOpType.mult)
        if use_gb:
            # general path: apply in f32, then fold gamma/beta, then copy
            xf = apool.tile([P, H], F32, tag="xf", bufs=2, name="xf")
            nc.scalar.activation(xf, a, AF.Identity, bias=nm, scale=rsig)
            nc.vector.tensor_mul(xf, xf, g_bc)
            nc.vector.tensor_add(xf, xf, b_bc)
            nc.vector.tensor_copy(dst, xf)
        else:
            nc.scalar.activation(dst, a, AF.Identity, bias=nm, scale=rsig)

    # ================ phase A: QT/KT/V ================
    with ExitStack() as phase_a:
        psA = phase_a.enter_context(tc.tile_pool(name="psA", bufs=1, space="PSUM"))

        # ---- QT / KT:  out[m] = W[:, m-block]^T @ xT  ([H,S] layout) ----
        for (w_t, dst, b_sb, useb) in ((wq_t, qt_sb, bq_sb, use_bq),
                                       (wk_t, kt_sb, bk_sb, use_bk)):
            for m in range(HK):
                ps = psA.tile([P, S], F32, tag="qk", bufs=3, name="psqk")
                for k in range(HK):
                    nc.tensor.matmul(ps, lhsT=w_t[:, m, k, :],
                                     rhs=xTr_sb[:, k, :],
                                     start=(k == 0), stop=(k == HK - 1))
                if useb:
                    nc.scalar.activation(dst[:, m, :], ps, AF.Identity,
                                         bias=b_sb[:, m:m + 1])
                else:
                    # eviction on DVE: ACT must stay free for the exp chain
                    nc.vector.tensor_copy(dst[:, m, :], ps)

        # ---- V in [S,H] layout with ones column per head -> vt_sb ----
        vt_v = vt_sb.rearrange("p m (h c) -> p m h c", c=VW)
        for m in range(SQ):
            nc.vector.tensor_copy(
                vt_v[:, m, :, DH:DH + 1],
                ones_t.rearrange("p (h o) -> p h o", o=1))

        def v_tile(m):
            ps = psA.tile([P, H], F32, tag="v", bufs=2, name="psv")
            for k in range(HK):
                nc.tensor.matmul(ps[:, 0:512],
                                 lhsT=xTr_sb[:, k, m * P:(m + 1) * P],
                                 rhs=wv_t[:, k, 0:512],
                                 start=(k == 0), stop=(k == HK - 1))
            for k in range(HK):
                nc.tensor.matmul(ps[:, 512:H],
                                 lhsT=xTr_sb[:, k, m * P:(m + 1) * P],
                                 rhs=wv_t[:, k, 512:H],
                                 start=(k == 0), stop=(k == HK - 1))
            src = ps.rearrange("p (h d) -> p h d", d=DH)
            if use_bv:
                nc.vector.tensor_add(
                    src, src, bv_bc.rearrange("p (h d) -> p h d", d=DH))
            nc.vector.tensor_copy(vt_v[:, m, :, 0:DH], src)

        for m in range(SQ):
            v_tile(m)

    # ================ phase B: attention + Wo + LN1 + transpose ========
    with ExitStack() as phase_b:
        psB = phase_b.enter_context(tc.tile_pool(name="psB", bufs=1, space="PSUM"))

        # ---- attention, head pairs: heads 2hp/2hp+1 at partition bases
        # 0/64 of the kt/qt tiles -> score matmuls on PE quadrants; both
        # score outputs share one 2-bank psum tile so exp processes 1024
        # columns per ACT op. ----
        def scores_pair(hp):
            est_l = []
            for sk in range(SQ):
                ps_s = psB.tile([P, 2 * S], F32, tag="s", bufs=2, name="pss")
                for half in range(2):
                    pb = 64 * half
                    nc.tensor.matmul(
                        ps_s[:, half * S:(half + 1) * S],
                        lhsT=kt_sb[pb:pb + DH, hp, sk * P:(sk + 1) * P],
                        rhs=qt_sb[pb:pb + DH, hp, :],
                        start=True, stop=True)
                est = expp.tile([P, 2 * S], BF, tag="est", bufs=4, name="est")
                if use_mask:
                    nc.scalar.activation(est, ps_s, AF.Exp,
                                         bias=msk_sb[:, sk:sk + 1], scale=0.125)
                else:
                    nc.scalar.activation(est, ps_s, AF.Exp, scale=0.125)
                est_l.append(est)
            return est_l

        def ctx_pair(hp, est_l):
            ps_cs = []
            for half in range(2):
                h = 2 * hp + half
                ps_c = psB.tile([P, S], F32, tag="c", bufs=4, name="psc")
                for sk in range(SQ):
                    nc.tensor.matmul(ps_c[0:VW, :],
                                     lhsT=vt_sb[:, sk, h * VW:(h + 1) * VW],
                                     rhs=est_l[sk][:, half * S:(half + 1) * S],
                                     start=(sk == 0), stop=(sk == SQ - 1))
                ps_cs.append(ps_c)
            # softmax normalization: gather both sums rows, one GpSimd
            # partition-broadcast, one wide fast-reciprocal; fused
            # eviction applies the normalization.
            srow2 = smalls.tile([1, 2 * S], F32, tag="srow", bufs=2)
            for half in range(2):
                nc.vector.tensor_copy(srow2[:, half * S:(half + 1) * S],
                                      ps_cs[half][DH:VW, :])
            rb = rbp.tile([P, 2 * S], F32, tag="rb")
            nc.gpsimd.partition_broadcast(rb, srow2)
            nc.vector.reciprocal_approx_fast(rb, rb)
            for half in range(2):
                pb = 64 * half
                nc.vector.tensor_tensor(
                    ctxt_sb[pb:pb + DH, hp, :], ps_cs[half][0:DH, :],
                    rb[pb:pb + DH, half * S:(half + 1) * S],
                    mybir.AluOpType.mult)

        for hp in range(NH // 2):
            est_l = scores_pair(hp)
            ctx_pair(hp, est_l)

        # ---- Wo + residual + LN1, s-tile pipelined; the k=5 (last head
        # pair) matmul term runs at the end of each chain so the last
        # pair's normalization latency hides behind k=0..4 work. ----
        KORD = [0, 1, 2, 3, 4, 5]

        def wo_chain(m, korder):
            ps = psB.tile([P, 2 * S], F32, tag="s", bufs=2, name="psw")
            for i, k in enumerate(korder):
                nc.tensor.matmul(ps[:, 0:512],
                                 lhsT=ctxt_sb[:, k, m * P:(m + 1) * P],
                                 rhs=wo_t[:, k, 0:512],
                                 start=(i == 0), stop=(i == HK - 1))
            for i, k in enumerate(korder):
                nc.tensor.matmul(ps[:, 512:H],
                                 lhsT=ctxt_sb[:, k, m * P:(m + 1) * P],
                                 rhs=wo_t[:, k, 512:H],
                                 start=(i == 0), stop=(i == HK - 1))
            return ps[:, 0:H]

        ps_w = {}
        # m0/m1 k0..4 first (hiding pair-5 latency), then their k5 terms
        for m in (0, 1):
            ps = psB.tile([P, 2 * S], F32, tag="s", bufs=2, name="psw")
            ps_w[m] = ps
            for half, lo, hi in ((0, 0, 512), (1, 512, H)):
                for i, k in enumerate(KORD[:5]):
                    nc.tensor.matmul(ps[:, lo:hi],
                                     lhsT=ctxt_sb[:, k, m * P:(m + 1) * P],
                                     rhs=wo_t[:, k, lo:hi],
                                     start=(i == 0), stop=False)
        for m in (0, 1):
            for half, lo, hi in ((0, 0, 512), (1, 512, H)):
                nc.tensor.matmul(ps_w[m][:, lo:hi],
                                 lhsT=ctxt_sb[:, 5, m * P:(m + 1) * P],
                                 rhs=wo_t[:, 5, lo:hi],
                                 start=False, stop=True)
        for m in (0, 1):
            resid_layernorm_tile(ps_w[m][:, 0:H], x_sb[:, m, :],
                                 bo_bc if use_bo else None,
                                 x1_sb[:, m, :], ln1g_bc, ln1b_bc, use_ln1)
        for m in (2, 3):
            ps = wo_chain(m, KORD)
            resid_layernorm_tile(ps, x_sb[:, m, :],
                                 bo_bc if use_bo else None,
                                 x1_sb[:, m, :], ln1g_bc, ln1b_bc, use_ln1)

        # ---- x1 -> x1T via PE transposes (bf16, 1 cycle/row); psum
        # evictions ride ACT (idle after attention) so DVE stays free
        # for the LN stat chains. ----
        for m in range(SQ):
            for kb in range(HK):
                ps_t = psB.tile([P, P], BF, tag="c", bufs=4, name="pst")
                nc.tensor.transpose(
                    ps_t, x1_sb[:, m, kb * P:(kb + 1) * P], ident)
                nc.scalar.copy(
                    x1t_sb[:, kb, m * P:(m + 1) * P], ps_t)

    # ================ phase C: FFN ================
    with ExitStack() as phase_c:
        psD = phase_c.enter_context(tc.tile_pool(name="psD", bufs=1, space="PSUM"))

        # ---- FFN1: hmidT[f,:] = Wi[:,f]^T @ x1T, GELU on eviction ----
        for mf in range(FK):
            wi_t = wstream.tile([P, HK, P], BF, tag="wi", bufs=6, name="wi")
            nc.sync.dma_start(out=wi_t, in_=wi_d[mf])
            ps = psD.tile([P, S], F32, tag="f1", bufs=4, name="psf1")
            for k in range(HK):
                nc.tensor.matmul(ps, lhsT=wi_t[:, k, :], rhs=x1t_sb[:, k, :],
                                 start=(k == 0), stop=(k == HK - 1))
            if use_bi:
                nc.scalar.activation(hmidt_sb[:, mf, :], ps, AF.Gelu,
                                     bias=bi_sb[:, mf:mf + 1])
            else:
                nc.scalar.activation(hmidt_sb[:, mf, :], ps, AF.Gelu)

        # ---- FFN2 s-tile-outer against resident Wout; LN2 + out DMA of
        # tile m overlap tile m+1 matmuls ----
        for m in range(SQ):
            ps = psD.tile([P, H], F32, tag="f2", bufs=2, name="psf2")
            for lo, hi in ((0, 512), (512, H)):
                for k in range(FK):
                    nc.tensor.matmul(ps[:, lo:hi],
                                     lhsT=hmidt_sb[:, k, m * P:(m + 1) * P],
                                     rhs=wout_t[:, k, lo:hi],
                                     start=(k == 0), stop=(k == FK - 1))
            o = outp.tile([P, H], F32, tag="out", bufs=2, name="o")
            resid_layernorm_tile(ps, x1_sb[:, m, :],
                                 bout_bc if use_bout else None,
                                 o, ln2g_bc, ln2b_bc, use_ln2)
            nc.sync.dma_start(out=out_d[m * P:(m + 1) * P, :], in_=o)


_NC_CACHE = {}


def build_nc(flags):
    key = tuple(flags)
    if key not in _NC_CACHE:
        nc = bacc.Bacc("TRN2")
        with ExitStack() as ctx:
            tc = ctx.enter_context(tile.TileContext(nc))
            _emit(ctx, tc, flags)
        nc.compile()
        _NC_CACHE[key] = nc
    return _NC_CACHE[key]


def _pack_lhsT(A, mt):
    # A [in, mt*P] -> [mt, P, in//P, P] tiles: out[m, p, k, f] = A[P*k+p, P*m+f]
    kt = A.shape[0] // P
    return np.ascontiguousarray(
        A.reshape(kt, P, mt, P).transpose(2, 1, 0, 3))


def _bf(a):
    return np.ascontiguousarray(np.asarray(a).astype(NPBF))


def kernel(**inputs):
    hs = np.ascontiguousarray(np.asarray(inputs["hidden_states"], dtype=np.float32))
    eidx = np.asarray(inputs["expert_idx"]).astype(np.int64)
    mask = np.asarray(inputs["attention_mask"], dtype=np.float32)
    Wq = np.asarray(inputs["Wq"], dtype=np.float32)
    bq = np.asarray(inputs["bq"], dtype=np.float32)
    Wk = np.asarray(inputs["Wk"], dtype=np.float32)
    bk = np.asarray(inputs["bk"], dtype=np.float32)
    Wv = np.asarray(inputs["Wv"], dtype=np.float32)
    bv = np.asarray(inputs["bv"], dtype=np.float32)
    Wo = np.asarray(inputs["Wo"], dtype=np.float32)
    bo = np.asarray(inputs["bo"], dtype=np.float32)
    ln1_g = np.asarray(inputs["ln1_g"], dtype=np.float32)
    ln1_b = np.asarray(inputs["ln1_b"], dtype=np.float32)
    Wi = np.asarray(inputs["Wi"], dtype=np.float32)
    bi = np.asarray(inputs["bi"], dtype=np.float32)
    Wout = np.asarray(inputs["Wout"], dtype=np.float32)
    bout = np.asarray(inputs["bout"], dtype=np.float32)
    ln2_g = np.asarray(inputs["ln2_g"], dtype=np.float32)
    ln2_b = np.asarray(inputs["ln2_b"], dtype=np.float32)

    B = hs.shape[0]
    assert hs.shape == (B, S, H) and B == N_CORES

    use_bq = bool(np.any(bq))
    use_bk = bool(np.any(bk))
    use_bv = bool(np.any(bv))
    use_bo = bool(np.any(bo))
    use_bi = bool(np.any(bi))
    use_bout = bool(np.any(bout))
    use_mask = bool(np.any(mask))
    use_ln1 = bool(np.any(ln1_g != 1.0) or np.any(ln1_b))
    use_ln2 = bool(np.any(ln2_g != 1.0) or np.any(ln2_b))
    flags = (use_bq, use_bk, use_bv, use_bo, use_bi, use_bout,
             use_mask, use_ln1, use_ln2)

    nc = build_nc(flags)

    # per-expert packed weights, converted once and reused across cores
    packed = {}
    for e in set(int(v) for v in eidx):
        packed[e] = {
            "wq": _bf(_pack_lhsT(Wq[e], HK)),
            "wk": _bf(_pack_lhsT(Wk[e], HK)),
            "wv": _bf(Wv[e].reshape(HK, P, H)),
            "wo": _bf(Wo[e].reshape(HK, P, H)),
            "wi": _bf(_pack_lhsT(Wi[e], FK)),
            "wout": _bf(Wout[e].reshape(FK, P, H)),
        }

    in_maps = []
    for b in range(B):
        e = int(eidx[b])
        xb = hs[b]
        im = {
            "x": _bf(xb),
            "xT": _bf(xb.T),
        }
        im.update(packed[e])
        if use_bq:
            im["bq"] = np.ascontiguousarray(bq[e].reshape(HK, P).T)
        if use_bk:
            im["bk"] = np.ascontiguousarray(bk[e].reshape(HK, P).T)
        if use_bv:
            im["bv"] = bv[e]
        if use_bo:
            im["bo"] = bo[e]
        if use_bi:
            im["bi"] = np.ascontiguousarray(bi[e].reshape(FK, P).T)
        if use_bout:
            im["bout"] = bout[e]
        if use_mask:
            im["msk"] = np.ascontiguousarray(mask[b, 0, 0, :].reshape(SQ, P).T)
        if use_ln1:
            im["ln1g"] = ln1_g
            im["ln1b"] = ln1_b
        if use_ln2:
            im["ln2g"] = ln2_g
            im["ln2b"] = ln2_b
        in_maps.append(im)

    from concourse.bass_utils import run_bass_kernel_spmd
    res = run_bass_kernel_spmd(nc, in_maps, core_ids=list(range(N_CORES)),
                               **RUN_KWARGS)
    global LAST_RESULTS
    LAST_RESULTS = res
    out = np.stack([res.results[b]["out"] for b in range(B)], axis=0)
    return out.astype(np.float32)


RUN_KWARGS = {}
LAST_RESULTS = None


if __name__ == "__main__":
    rng = np.random.default_rng(0)
    demo = {
        "hidden_states": rng.standard_normal((8, S, H), dtype=np.float32),
        "expert_idx": rng.integers(0, 4, size=8).astype(np.int32),
        "attention_mask": np.zeros((8, 1, 1, S), np.float32),
        "Wq": 0.02 * rng.standard_normal((4, H, H), dtype=np.float32),
        "bq": np.zeros((4, H), np.float32),
        "Wk": 0.02 * rng.standard_normal((4, H, H), dtype=np.float32),
        "bk": np.zeros((4, H), np.float32),
        "Wv": 0.02 * rng.standard_normal((4, H, H), dtype=np.float32),
        "bv": np.zeros((4, H), np.float32),
        "Wo": 0.02 * rng.standard_normal((4, H, H), dtype=np.float32),
        "bo": np.zeros((4, H), np.float32),
        "ln1_g": np.ones((H,), np.float32),
        "ln1_b": np.zeros((H,), np.float32),
        "Wi": 0.02 * rng.standard_normal((4, H, FF), dtype=np.float32),
        "bi": np.zeros((4, FF), np.float32),
        "Wout": 0.02 * rng.standard_normal((4, FF, H), dtype=np.float32),
        "bout": np.zeros((4, H), np.float32),
        "ln2_g": np.ones((H,), np.float32),
        "ln2_b": np.zeros((H,), np.float32),
    }
    out = kernel(**demo)
    print("out", out.shape, out.dtype, float(np.abs(out).mean()))
